# revision 1
# baseline (speedup 1.0000x reference)
"""Trainium2 Bass kernel for nn_BinaryLinear: out = sign(x @ sign(W).T + bias).

Strategy
--------
Data-parallel over the 8192-token dim: each of the 8 cores gets 1024 tokens
and the full weight matrix.

On-chip compute (per core) is the NT GEMM z.T = sign(W) @ x.T on the
TensorEngine with the contraction (in_features) on the partition dim:

  psum[outf, tok] = sum_k w_b_T[k, outf] * x_T[k, tok]

Both operands are pre-transposed on the host (pure layout prep) so every DMA
is contiguous-per-partition. Precision/speed: x is split as

  x ~= fp16(x) + 2^-6 * e4m3((x - fp16(x)) * 2^6)        (~15-16 mantissa bits)

The hi half runs as regular fp16 matmuls (1 PE cycle/row). The lo half runs
as fp8e4m3 DoubleRow matmuls (2x FLOPs per instruction, 256-deep contraction)
with the 2^-6 scale folded into the fp8 weights (+-2^-6 is exact in e4m3),
so BOTH halves accumulate into the same fp32 PSUM group with no epilogue
combine. Combined error lands at the fp32 reference's own accumulation-error
scale. fp32 matmul would be 4 cycles/row; a bf16 hi+lo split is 2 cycles/row;
this scheme is ~1.5.

sign(W) is computed on-chip (ScalarE Sign: fp32 -> fp16 +-1, then VectorE
*2^-6 -> e4m3). The epilogue fuses bias-add + sign + PSUM->SBUF in one
ScalarE activation (bias is per-partition in the z.T layout). Output is
written as z.T [out_features, tokens] per core and untransposed on the host.

Within each PSUM group all fp16 MMs run before the DoubleRow MMs, so the lo
x data is needed one hi-phase later than the hi data and the single serial
gpsimd DMA stream (hi chunks in k order, then lo chunks) stays ahead of the
PE after the first iteration. Measured on HW: ~710us per core (PE matmul
roofline for this scheme is ~654us).
"""

import numpy as np

import concourse.tile as tile
import concourse.mybir as mybir
from concourse import bacc
from concourse.bass_utils import run_bass_kernel_spmd
from concourse.tile_rust import add_dep_helper

N_CORES = 8
N_TOK = 8192
D_IN = 4096
D_OUT = 4096
P = 128
T = N_TOK // N_CORES  # 1024 tokens per core
KT = D_IN // P  # 32 contraction tiles
KP = KT // 2  # 16 DoubleRow k-pairs
MT = D_OUT // P  # 32 out-feature tiles
M2 = 2  # m-tiles per cached W block (256 outf cols)
MB = MT // M2  # 16 W blocks
TB = 512  # token block (one PSUM bank of fp32)
NB = T // TB  # 2 token blocks per core
LO_SCALE = 2.0 ** 6  # host-side scale on the fp8 residual; inverse on weights

F32 = mybir.dt.float32
FP16 = mybir.dt.float16
FP8 = mybir.dt.float8e4
SIGN = mybir.ActivationFunctionType.Sign
DR = mybir.MatmulPerfMode.DoubleRow
E4M3 = mybir.dt.np(FP8)

_nc_cache = None


def build():
    """Build + compile the per-core Bass/Tile module (SPMD: same on all cores)."""
    global _nc_cache
    if _nc_cache is not None:
        return _nc_cache
    nc = bacc.Bacc("TRN2", target_bir_lowering=False, debug=False, num_devices=N_CORES)
    xhi_d = nc.dram_tensor("x_hi_t", [D_IN, T], FP16, kind="ExternalInput").ap()
    xlo_d = nc.dram_tensor("x_lo8_t", [D_IN, T], FP8, kind="ExternalInput").ap()
    w_d = nc.dram_tensor("w_t", [D_IN, D_OUT], F32, kind="ExternalInput").ap()
    b_d = nc.dram_tensor("bias", [D_OUT], F32, kind="ExternalInput").ap()
    out_d = nc.dram_tensor("out_t", [D_OUT, T], F32, kind="ExternalOutput").ap()

    with tile.TileContext(nc) as tc:
        with (
            tc.tile_pool(name="x", bufs=1) as x_pool,
            tc.tile_pool(name="wstage", bufs=8) as wstage_pool,
            tc.tile_pool(name="wsb", bufs=3) as w_pool,
            tc.tile_pool(name="bias", bufs=1) as b_pool,
            tc.tile_pool(name="out", bufs=6) as out_pool,
            tc.tile_pool(name="psum", bufs=8, space="PSUM") as psum_pool,
        ):
            def convert_w_block(mb):
                # Stage a [D_IN, 256] W column block; convert to
                # sign() in fp16 (+-1) and e4m3 (+-2^-6).
                wsb_hi = w_pool.tile([P, KT, M2 * P], FP16, tag="wsb_hi",
                                     name=f"wsb_hi_{mb}")
                wsb_lo = w_pool.tile([P, KT, M2 * P], FP8, tag="wsb_lo",
                                     name=f"wsb_lo_{mb}")
                for k in range(KT):
                    wstage = wstage_pool.tile([P, M2 * P], F32, tag="wstage",
                                              name=f"wstage_{mb}_{k}")
                    nc.sync.dma_start(
                        wstage[:],
                        w_d[k * P : (k + 1) * P, mb * M2 * P : (mb + 1) * M2 * P],
                    )
                    nc.scalar.activation(wsb_hi[:, k, :], wstage[:], SIGN)
                    nc.vector.tensor_scalar_mul(
                        wsb_lo[:, k, :], wsb_hi[:, k, :], 1.0 / LO_SCALE
                    )
                return wsb_hi, wsb_lo

            # mb0's W conversion is emitted first so its ScalarE/VectorE ops
            # are not queued behind anything on those engines.
            wsb_cache = {0: convert_w_block(0)}

            # Resident x, chunked per k-tile (full token width) so matmuls
            # depend on exactly the chunk they read, all on the gpsimd queue
            # (the sync queue streams W).
            # The tail half of the hi chunks and all lo chunks are gated on
            # early mb0 compute (add_dep_helper below) so the chunks the PE
            # needs first get the full DMA-ring bandwidth instead of
            # fair-sharing it with everything in flight.
            xhi = []
            xlo8 = []
            hi_tail_dmas = []
            lo_dmas = []
            for ko in range(KT):
                th = x_pool.tile([P, T], FP16, tag=f"xh_{ko}", name=f"xh_{ko}")
                dma = nc.gpsimd.dma_start(th[:], xhi_d[ko * P : (ko + 1) * P, :])
                if ko >= 16:
                    hi_tail_dmas.append(dma.ins)
                xhi.append(th)
            for t2 in range(KP):
                tl = x_pool.tile([P, 2, T], FP8, tag=f"xl_{t2}", name=f"xl_{t2}")
                for j in range(2):
                    ko = 2 * t2 + j
                    dma = nc.gpsimd.dma_start(
                        tl[:, j, :], xlo_d[ko * P : (ko + 1) * P, :]
                    )
                    lo_dmas.append(dma.ins)
                xlo8.append(tl)
            gate_hi = gate_lo = None  # mb0 MMs at k=4 / k=12
            # bias, outf-partition-major: bias_sb[p, mo] = bias[mo*128 + p]
            bias_sb = b_pool.tile([P, MT], F32, tag="bias")
            nc.sync.dma_start(bias_sb[:], b_d.rearrange("(mo p) -> p mo", p=P))

            for mb in range(MB):
                if mb not in wsb_cache:
                    wsb_cache[mb] = convert_w_block(mb)
                wsb_hi, wsb_lo = wsb_cache.pop(mb)

                # Both token-blocks interleaved inside the k loop: each
                # weight load (LDWEIGHTS) feeds two 512-col matmuls, so the
                # weight-load stream is fully hidden. 4 PSUM groups live
                # (M2 x NB) = 4 banks; bufs=8 double-buffers across mb.
                nsls = [slice(n * TB, (n + 1) * TB) for n in range(NB)]
                psums = {
                    (mi, n): psum_pool.tile([P, TB], F32, tag="psum",
                                            name=f"ps_{mb}_{n}_{mi}")
                    for mi in range(M2)
                    for n in range(NB)
                }
                for k in range(KT):
                    for mi in range(M2):
                        msl = slice(mi * P, (mi + 1) * P)
                        for n in range(NB):
                            mm = nc.tensor.matmul(
                                psums[(mi, n)][:],
                                wsb_hi[:, k, msl],
                                xhi[k][:, nsls[n]],
                                start=(k == 0),
                                stop=False,
                            )
                            if mb == 0 and mi == M2 - 1 and n == NB - 1:
                                if k == 4:
                                    gate_hi = mm.ins
                                elif k == 12:
                                    gate_lo = mm.ins
                for t in range(KP):
                    for mi in range(M2):
                        msl = slice(mi * P, (mi + 1) * P)
                        for n in range(NB):
                            nc.tensor.matmul(
                                psums[(mi, n)][:],
                                wsb_lo[:, 2 * t : 2 * t + 2, msl],
                                xlo8[t][:, :, nsls[n]],
                                start=False,
                                stop=(t == KP - 1),
                                perf_mode=DR,
                            )
                for mi in range(M2):
                    m = mb * M2 + mi
                    for n in range(NB):
                        osb = out_pool.tile([P, TB], F32, tag="osb",
                                            name=f"osb_{mb}_{n}_{mi}")
                        nc.scalar.activation(
                            osb[:], psums[(mi, n)][:], SIGN,
                            bias=bias_sb[:, m : m + 1],
                        )
                        nc.sync.dma_start(
                            out_d[m * P : (m + 1) * P, nsls[n]], osb[:]
                        )
    nc.compile()
    _nc_cache = nc
    return nc


def prep_in_maps(x, weight, bias):
    """Host-side layout prep: fp16/fp8 split of x, transposes, token shards."""
    x = np.asarray(x, dtype=np.float32)
    weight = np.asarray(weight, dtype=np.float32)
    bias = np.asarray(bias, dtype=np.float32)

    x_hi = x.astype(np.float16)
    x_lo8 = ((x - x_hi.astype(np.float32)) * LO_SCALE).astype(E4M3)
    xhi_t = np.ascontiguousarray(x_hi.T)  # [D_IN, N_TOK]
    xlo_t = np.ascontiguousarray(x_lo8.T)
    w_t = np.ascontiguousarray(weight.T)  # [D_IN, D_OUT]

    in_maps = []
    for c in range(N_CORES):
        sl = slice(c * T, (c + 1) * T)
        in_maps.append(
            {
                "x_hi_t": np.ascontiguousarray(xhi_t[:, sl]),
                "x_lo8_t": np.ascontiguousarray(xlo_t[:, sl]),
                "w_t": w_t,
                "bias": bias,
            }
        )
    return in_maps


def run(x, weight, bias, **spmd_kwargs):
    """Run on the 8 cores; returns (full_output, BassKernelResults)."""
    nc = build()
    in_maps = prep_in_maps(x, weight, bias)
    res = run_bass_kernel_spmd(nc, in_maps, core_ids=list(range(N_CORES)), **spmd_kwargs)
    out = np.empty((N_TOK, D_OUT), dtype=np.float32)
    for c in range(N_CORES):
        out[c * T : (c + 1) * T, :] = res.results[c]["out_t"].T
    return out, res


def kernel(x, weight, bias):
    out, _ = run(x, weight, bias)
    return out



# revision 2
# speedup vs baseline: 1.0253x; 1.0253x over previous
"""Trainium2 Bass kernel for nn_BinaryLinear: out = sign(x @ sign(W).T + bias).

Strategy
--------
Data-parallel over the 8192-token dim: each of the 8 cores gets 1024 tokens
and the full weight matrix.

On-chip compute (per core) is the NT GEMM z.T = sign(W) @ x.T on the
TensorEngine with the contraction (in_features) on the partition dim:

  psum[outf, tok] = sum_k w[k, outf] * x[k, tok]

Precision/speed: every matmul runs as an fp8e4 DoubleRow matmul (2x FLOPs
per instruction, 256-deep contraction, 0.5 PE cycles per output row). x is
expanded host-side into THREE e4m3 planes of u = 32*x:

  x1 = e4m3(u); x2 = e4m3(u - x1); x3 = e4m3(u - x1 - x2)

and the weights are shipped as a single e4m3 array w5 = sign(W)*2^-5
(+-2^-5 is exact in e4m3), so

  sum_p (x1+x2+x3)[k,tok] * w5[k,outf] ~= x[k,tok]*sign(W)[outf,k]

with ~15 effective mantissa bits (measured on the real inputs: 227/33.5M
sign flips, rel_err 5.0e-3, vs the 2e-2 budget). All three planes share the
same stationary weights and accumulate into the same fp32 PSUM group, so
there is no epilogue combine. PE roofline for this scheme is ~444us/core
(3072 DR matmuls x 256 cyc x 0.5ns x ~1.13 DR adder penalty) vs 655us for
the previous fp16+fp8 hi/lo split.

The fp8 weights also shrink W DMA 4x (16MB vs 64MB fp32) and remove the
on-chip Sign/scale conversion (ScalarE/VectorE now only run the epilogue).
The epilogue fuses bias-add + sign + PSUM->SBUF in one ScalarE activation
(bias is per-partition in the z.T layout). Output is written as z.T
[out_features, tokens] per core and untransposed on the host.
"""

import numpy as np

import concourse.tile as tile
import concourse.mybir as mybir
from concourse import bacc
from concourse.bass_utils import run_bass_kernel_spmd

N_CORES = 8
N_TOK = 8192
D_IN = 4096
D_OUT = 4096
P = 128
T = N_TOK // N_CORES  # 1024 tokens per core
KT = D_IN // P  # 32 contraction tiles
KP = KT // 2  # 16 DoubleRow k-pairs
MT = D_OUT // P  # 32 out-feature tiles
M2 = 2  # m-tiles per cached W block (256 outf cols)
MB = MT // M2  # 16 W blocks
TB = 512  # token block (one PSUM bank of fp32)
NB = T // TB  # 2 token blocks per core
NPLANES = 3
W_SCALE = 2.0 ** -5  # weight magnitude; x planes carry u = x / W_SCALE

F32 = mybir.dt.float32
FP8 = mybir.dt.float8e4
SIGN = mybir.ActivationFunctionType.Sign
DR = mybir.MatmulPerfMode.DoubleRow
E4M3 = mybir.dt.np(FP8)

_nc_cache = None


def build():
    """Build + compile the per-core Bass/Tile module (SPMD: same on all cores)."""
    global _nc_cache
    if _nc_cache is not None:
        return _nc_cache
    nc = bacc.Bacc("TRN2", target_bir_lowering=False, debug=False, num_devices=N_CORES)
    xp_d = [
        nc.dram_tensor(f"x_p{i}_t", [D_IN, T], FP8, kind="ExternalInput").ap()
        for i in range(NPLANES)
    ]
    w_d = nc.dram_tensor("w5_t", [D_IN, D_OUT], FP8, kind="ExternalInput").ap()
    b_d = nc.dram_tensor("bias", [D_OUT], F32, kind="ExternalInput").ap()
    out_d = nc.dram_tensor("out_t", [D_OUT, T], F32, kind="ExternalOutput").ap()

    with tile.TileContext(nc) as tc:
        with (
            tc.tile_pool(name="x", bufs=1) as x_pool,
            tc.tile_pool(name="wsb", bufs=4) as w_pool,
            tc.tile_pool(name="bias", bufs=1) as b_pool,
            tc.tile_pool(name="out", bufs=6) as out_pool,
            tc.tile_pool(name="psum", bufs=8, space="PSUM") as psum_pool,
        ):
            # Resident x planes as DoubleRow k-pair tiles [P, 2, T], all on
            # the gpsimd queue (the sync queue streams W). Emission order =
            # queue order: plane 0 first, matching PE consumption order.
            xp = {}
            for pl in range(NPLANES):
                for t in range(KP):
                    tl = x_pool.tile([P, 2, T], FP8, tag=f"x{pl}_{t}",
                                     name=f"x{pl}_{t}")
                    for j in range(2):
                        ko = 2 * t + j
                        nc.gpsimd.dma_start(
                            tl[:, j, :], xp_d[pl][ko * P : (ko + 1) * P, :]
                        )
                    xp[(pl, t)] = tl

            # bias, outf-partition-major: bias_sb[p, mo] = bias[mo*128 + p]
            bias_sb = b_pool.tile([P, MT], F32, tag="bias")
            nc.sync.dma_start(bias_sb[:], b_d.rearrange("(mo p) -> p mo", p=P))

            for mb in range(MB):
                # Stage the [D_IN, 256] W column block, already e4m3 +-2^-5.
                # Layout [P, KT, M2*P]: k-pair slices [:, 2t:2t+2, msl] feed
                # DoubleRow matmuls directly. bufs=4 prefetches ahead.
                w5 = w_pool.tile([P, KT, M2 * P], FP8, tag="w5", name=f"w5_{mb}")
                for k in range(KT):
                    nc.sync.dma_start(
                        w5[:, k, :],
                        w_d[k * P : (k + 1) * P, mb * M2 * P : (mb + 1) * M2 * P],
                    )

                # Both token-blocks interleaved inside the k loop: each
                # weight load feeds two 512-col matmuls. 4 PSUM groups live
                # (M2 x NB) = 4 banks; bufs=8 double-buffers across mb.
                nsls = [slice(n * TB, (n + 1) * TB) for n in range(NB)]
                psums = {
                    (mi, n): psum_pool.tile([P, TB], F32, tag="psum",
                                            name=f"ps_{mb}_{n}_{mi}")
                    for mi in range(M2)
                    for n in range(NB)
                }
                for pl in range(NPLANES):
                    for t in range(KP):
                        for mi in range(M2):
                            msl = slice(mi * P, (mi + 1) * P)
                            for n in range(NB):
                                nc.tensor.matmul(
                                    psums[(mi, n)][:],
                                    w5[:, 2 * t : 2 * t + 2, msl],
                                    xp[(pl, t)][:, :, nsls[n]],
                                    start=(pl == 0 and t == 0),
                                    stop=(pl == NPLANES - 1 and t == KP - 1),
                                    perf_mode=DR,
                                )
                for mi in range(M2):
                    m = mb * M2 + mi
                    for n in range(NB):
                        osb = out_pool.tile([P, TB], F32, tag="osb",
                                            name=f"osb_{mb}_{n}_{mi}")
                        nc.scalar.activation(
                            osb[:], psums[(mi, n)][:], SIGN,
                            bias=bias_sb[:, m : m + 1],
                        )
                        nc.sync.dma_start(
                            out_d[m * P : (m + 1) * P, nsls[n]], osb[:]
                        )
    nc.compile()
    _nc_cache = nc
    return nc


def prep_in_maps(x, weight, bias):
    """Host-side layout prep: 3-plane e4m3 split of u=32x, transposes, shards."""
    x = np.asarray(x, dtype=np.float32)
    weight = np.asarray(weight, dtype=np.float32)
    bias = np.asarray(bias, dtype=np.float32)

    u = np.clip(x * (1.0 / W_SCALE), -240.0, 240.0)
    x1 = u.astype(E4M3)
    r = u - x1.astype(np.float32)
    x2 = r.astype(E4M3)
    r -= x2.astype(np.float32)
    x3 = r.astype(E4M3)
    planes = [np.ascontiguousarray(p.T) for p in (x1, x2, x3)]  # [D_IN, N_TOK]

    w5_t = np.ascontiguousarray(
        (np.sign(weight) * np.float32(W_SCALE)).astype(E4M3).T
    )  # [D_IN, D_OUT]

    in_maps = []
    for c in range(N_CORES):
        sl = slice(c * T, (c + 1) * T)
        m = {f"x_p{i}_t": np.ascontiguousarray(planes[i][:, sl])
             for i in range(NPLANES)}
        m["w5_t"] = w5_t
        m["bias"] = bias
        in_maps.append(m)
    return in_maps


def run(x, weight, bias, **spmd_kwargs):
    """Run on the 8 cores; returns (full_output, BassKernelResults)."""
    nc = build()
    in_maps = prep_in_maps(x, weight, bias)
    res = run_bass_kernel_spmd(nc, in_maps, core_ids=list(range(N_CORES)), **spmd_kwargs)
    out = np.empty((N_TOK, D_OUT), dtype=np.float32)
    for c in range(N_CORES):
        out[c * T : (c + 1) * T, :] = res.results[c]["out_t"].T
    return out, res


def kernel(x, weight, bias):
    out, _ = run(x, weight, bias)
    return out


# revision 3
# speedup vs baseline: 1.3824x; 1.3482x over previous
"""Trainium2 Bass kernel for nn_BinaryLinear: out = sign(x @ sign(W).T + bias).

Strategy
--------
Data-parallel over the 8192-token dim: each of the 8 cores gets 1024 tokens
and the full weight matrix.

On-chip compute (per core) is the NT GEMM z.T = sign(W) @ x.T on the
TensorEngine with the contraction (in_features) on the partition dim:

  psum[outf, tok] = sum_k w[k, outf] * x[k, tok]

Precision/speed: the moving operand x is kept in float32r ("fp32 reduced":
the PE reads 4-byte fp32 and truncates to FP22 = e8m13). HW-measured, an
fp32r matmul with free dim 512 issues at ~227ns — the same ~1 row/cycle
rate as fp16/bf16/fp8 (every matmul instruction on this part streams one
moving row per cycle at 2.4GHz regardless of dtype; fp8 DoubleRow doubles
the contraction per instruction but fp8's 3-bit mantissa then needs 3
planes, which costs MORE instructions than one 13-bit fp32r pass). A
single fp32r pass is the instruction-count optimum:

  32 k-tiles x 32 outf-tiles x 2 token-blocks = 2048 matmuls x ~227ns
  ~= 465us/core, vs 48 instr/block (~660us) for any fp16+fp8 or 3xfp8
  scheme.

Accuracy: weights +-1 are exact in FP22; x truncated to 13 mantissa bits
gives (measured on the real inputs) 289/33.5M sign flips, rel_err 5.9e-3
vs the 2e-2 budget.

W ships as e4m3 +-1 [D_IN, D_OUT] (16MB vs 64MB fp32) and is upconverted
per 128-column block to an fp32r SBUF tile by the otherwise-idle VectorE.
fp32r weights must go through the self-loading matmul (standalone
ldweights is broken for 4-byte dtypes) — nc.tensor.matmul handles it.

Schedule: blocks 0 and 1 run k-major interleaved (4 PSUM groups) so the
PE tracks the incoming x stream instead of stalling on the last k-tile of
block 0; x k-tiles stream on two queues (even k on gpsimd, odd k on sync).
Remaining blocks run sequentially, PSUM double-buffered. The epilogue
(bias-add + sign + PSUM->SBUF in one ScalarE activation, bias is
per-partition in the z.T layout) and the output DMAs live on the scalar
queue so they never queue behind the W/x streams. Output is written as
z.T [out_features, tokens] per core and untransposed on the host.
"""

import numpy as np

import concourse.tile as tile
import concourse.mybir as mybir
from concourse import bacc
from concourse.bass_utils import run_bass_kernel_spmd

N_CORES = 8
N_TOK = 8192
D_IN = 4096
D_OUT = 4096
P = 128
T = N_TOK // N_CORES  # 1024 tokens per core
KT = D_IN // P  # 32 contraction tiles
MT = D_OUT // P  # 32 out-feature tiles (= W blocks)
TB = 512  # token block (one PSUM bank of fp32)
NB = T // TB  # 2 token blocks per core
PAIR = 2  # leading blocks run k-major interleaved

F32 = mybir.dt.float32
F32R = mybir.dt.float32r
FP8 = mybir.dt.float8e4
SIGN = mybir.ActivationFunctionType.Sign
E4M3 = mybir.dt.np(FP8)

_nc_cache = None


def build():
    """Build + compile the per-core Bass/Tile module (SPMD: same on all cores)."""
    global _nc_cache
    if _nc_cache is not None:
        return _nc_cache
    nc = bacc.Bacc("TRN2", target_bir_lowering=False, debug=False, num_devices=N_CORES)
    x_d = nc.dram_tensor("x_t", [D_IN, T], F32R, kind="ExternalInput").ap()
    w_d = nc.dram_tensor("w8_t", [D_IN, D_OUT], FP8, kind="ExternalInput").ap()
    b_d = nc.dram_tensor("bias", [D_OUT], F32, kind="ExternalInput").ap()
    out_d = nc.dram_tensor("out_t", [D_OUT, T], F32, kind="ExternalOutput").ap()

    # [p, kt, m] view of W: one 3D DMA per 128-outf block.
    w_v = w_d.rearrange("(kt p) m -> p kt m", p=P)

    with tile.TileContext(nc) as tc:
        with (
            tc.tile_pool(name="x", bufs=1) as x_pool,
            tc.tile_pool(name="w8", bufs=3) as w8_pool,
            tc.tile_pool(name="w32", bufs=3) as w32_pool,
            tc.tile_pool(name="bias", bufs=1) as b_pool,
            tc.tile_pool(name="out", bufs=6) as out_pool,
            tc.tile_pool(name="psum", bufs=8, space="PSUM") as psum_pool,
        ):
            # W blocks 0,1 first on the sync queue (block 0's convert gates
            # the very first matmul), then bias, then odd x k-tiles, then the
            # remaining W blocks. Even x k-tiles stream on gpsimd.
            def stage_w8(b):
                w8 = w8_pool.tile([P, KT, P], FP8, tag="w8", name=f"w8_{b}")
                nc.sync.dma_start(w8[:], w_v[:, :, b * P : (b + 1) * P])
                return w8

            w8_tiles = {b: stage_w8(b) for b in range(PAIR)}

            bias_sb = b_pool.tile([P, MT], F32, tag="bias")
            nc.sync.dma_start(bias_sb[:], b_d.rearrange("(mo p) -> p mo", p=P))

            xk = []
            for k in range(KT):
                tl = x_pool.tile([P, T], F32R, tag=f"x_{k}", name=f"x_{k}")
                xk.append(tl)
            for k in range(0, KT, 2):  # even k: gpsimd queue
                nc.gpsimd.dma_start(xk[k][:], x_d[k * P : (k + 1) * P, :])
            for k in range(1, KT, 2):  # odd k: sync queue
                nc.sync.dma_start(xk[k][:], x_d[k * P : (k + 1) * P, :])
            for b in range(PAIR, MT):
                w8_tiles[b] = stage_w8(b)

            # VectorE upconvert, in block order (one whole-block copy each).
            w32_tiles = {}
            for b in range(MT):
                w32 = w32_pool.tile([P, KT, P], F32R, tag="w32", name=f"w32_{b}")
                nc.vector.tensor_copy(w32[:], w8_tiles[b][:])
                w32_tiles[b] = w32

            nsls = [slice(n * TB, (n + 1) * TB) for n in range(NB)]

            def epilogue(b, psums):
                for n in range(NB):
                    osb = out_pool.tile([P, TB], F32, tag="osb",
                                        name=f"osb_{b}_{n}")
                    nc.scalar.activation(
                        osb[:], psums[n][:], SIGN,
                        bias=bias_sb[:, b : b + 1],
                    )
                    nc.scalar.dma_start(out_d[b * P : (b + 1) * P, nsls[n]], osb[:])

            def mk_psums(b):
                return [
                    psum_pool.tile([P, TB], F32, tag="psum", name=f"ps_{b}_{n}")
                    for n in range(NB)
                ]

            # Blocks 0..PAIR-1: k-major interleaved so the PE consumes x
            # k-tiles at ~stream rate instead of stalling on the last one.
            pair_ps = {b: mk_psums(b) for b in range(PAIR)}
            for k in range(KT):
                for b in range(PAIR):
                    for n in range(NB):
                        nc.tensor.matmul(
                            pair_ps[b][n][:],
                            w32_tiles[b][:, k, :],
                            xk[k][:, nsls[n]],
                            start=(k == 0),
                            stop=(k == KT - 1),
                        )
            for b in range(PAIR):
                epilogue(b, pair_ps[b])

            for b in range(PAIR, MT):
                psums = mk_psums(b)
                for k in range(KT):
                    for n in range(NB):
                        nc.tensor.matmul(
                            psums[n][:],
                            w32_tiles[b][:, k, :],
                            xk[k][:, nsls[n]],
                            start=(k == 0),
                            stop=(k == KT - 1),
                        )
                epilogue(b, psums)
    nc.compile()
    _nc_cache = nc
    return nc


def prep_in_maps(x, weight, bias):
    """Host-side layout prep: transposes, fp8 sign weights, token shards."""
    x = np.asarray(x, dtype=np.float32)
    weight = np.asarray(weight, dtype=np.float32)
    bias = np.asarray(bias, dtype=np.float32)

    x_t = np.ascontiguousarray(x.T)  # [D_IN, N_TOK]
    w8_t = np.ascontiguousarray(np.sign(weight).astype(E4M3).T)  # [D_IN, D_OUT]

    in_maps = []
    for c in range(N_CORES):
        sl = slice(c * T, (c + 1) * T)
        in_maps.append(
            {
                "x_t": np.ascontiguousarray(x_t[:, sl]),
                "w8_t": w8_t,
                "bias": bias,
            }
        )
    return in_maps


def run(x, weight, bias, **spmd_kwargs):
    """Run on the 8 cores; returns (full_output, BassKernelResults)."""
    nc = build()
    in_maps = prep_in_maps(x, weight, bias)
    res = run_bass_kernel_spmd(nc, in_maps, core_ids=list(range(N_CORES)), **spmd_kwargs)
    out = np.empty((N_TOK, D_OUT), dtype=np.float32)
    for c in range(N_CORES):
        out[c * T : (c + 1) * T, :] = res.results[c]["out_t"].T
    return out, res


def kernel(x, weight, bias):
    out, _ = run(x, weight, bias)
    return out


# revision 4
# speedup vs baseline: 1.4167x; 1.0249x over previous
"""Trainium2 Bass kernel for nn_BinaryLinear: out = sign(x @ sign(W).T + bias).

Strategy
--------
Data-parallel over the 8192-token dim: each of the 8 cores gets 1024 tokens
and the full weight matrix.

On-chip compute (per core) is the NT GEMM z.T = sign(W) @ x.T on the
TensorEngine with the contraction (in_features) on the partition dim:

  psum[outf, tok] = sum_k w[k, outf] * x[k, tok]

Precision/speed: the moving operand x is kept in float32r ("fp32 reduced"):
the PE reads 4-byte fp32 and rounds to 11 explicit mantissa bits, RNE
(probed on HW with one-hot weights). HW-measured, an fp32r matmul with
free dim 512 issues at ~224ns — the same ~1 row/cycle rate as fp16/bf16/
fp8 (every matmul on this part streams one moving row per cycle at 2.4GHz
regardless of dtype; fp8 DoubleRow doubles contraction per instruction but
fp8's 3-bit mantissa then needs 3 planes = more instructions than one
12-bit fp32r pass). A single fp32r pass is the instruction-count optimum:

  32 k-tiles x 32 outf-tiles x 2 token-blocks = 2048 matmuls x ~224ns
  ~= 460us/core, vs 48 instr/block (~660us) for any fp16+fp8 / 3xfp8
  scheme. Weights +-1 are exact in any dtype; measured end-to-end
  rel_err 1.1e-2 vs the 2e-2 budget.

Layout/DMA: every transfer is contiguous per partition (strided-gather
DMAs on this part are descriptor-bound at ~4.6ns per element):
  - W ships as e4m3 +-1 pre-arranged host-side into per-block slabs
    w8[p, b, kt, m] so block b stages with ONE 512KB DMA (4KB/partition),
    then the otherwise-idle VectorE upconverts it to an fp32r SBUF tile
    (fp32r weights must go through the self-loading matmul; standalone
    ldweights is broken for 4-byte dtypes).
  - bias ships pre-transposed [128, MT] (the naive "(mo p) -> p mo"
    rearrange DMA is 4096 4-byte descriptors = 21.7us of queue time).
  - output is written as e4m3 (sign is +-1, exact) z.T [out_f, tok] and
    untransposed/upcast on the host.

Schedule: x k-tiles stream on two queues (even k on gpsimd, odd k
interleaved with the first W slabs on sync). The first four blocks run
k-major interleaved across all 8 PSUM banks — blocks 2,3 join at k>=8 and
sweep their k<8 tail afterwards — so the PE tracks the incoming x stream
instead of stalling on the last k-tile of block 0. Remaining blocks run
sequentially (2 banks each, 4-deep pipelined). The epilogue (bias-add +
sign + PSUM->SBUF in one ScalarE activation; bias is per-partition in the
z.T layout) plus output DMAs live on the scalar queue so they never queue
behind the W/x streams.
"""

import numpy as np

import concourse.tile as tile
import concourse.mybir as mybir
from concourse import bacc
from concourse.bass_utils import run_bass_kernel_spmd

N_CORES = 8
N_TOK = 8192
D_IN = 4096
D_OUT = 4096
P = 128
T = N_TOK // N_CORES  # 1024 tokens per core
KT = D_IN // P  # 32 contraction tiles
MT = D_OUT // P  # 32 out-feature tiles (= W blocks)
TB = 512  # token block (one PSUM bank of fp32)
NB = T // TB  # 2 token blocks per core
QUAD = 4  # leading blocks run k-major interleaved (8 PSUM banks)
KSTAG = 8  # blocks 2,3 join the k-major loop here (their w32 lands late)

F32 = mybir.dt.float32
F32R = mybir.dt.float32r
FP8 = mybir.dt.float8e4
SIGN = mybir.ActivationFunctionType.Sign
E4M3 = mybir.dt.np(FP8)

_nc_cache = None


def build():
    """Build + compile the per-core Bass/Tile module (SPMD: same on all cores)."""
    global _nc_cache
    if _nc_cache is not None:
        return _nc_cache
    nc = bacc.Bacc("TRN2", target_bir_lowering=False, debug=False, num_devices=N_CORES)
    x_d = nc.dram_tensor("x_t", [D_IN, T], F32R, kind="ExternalInput").ap()
    # per-block weight slabs: w8[p, b, kt, m] = sign(W)[b*128+m, kt*128+p]
    w_d = nc.dram_tensor("w8_slab", [P, MT, KT, P], FP8, kind="ExternalInput").ap()
    b_d = nc.dram_tensor("bias_t", [P, MT], F32, kind="ExternalInput").ap()
    out_d = nc.dram_tensor("out_t", [D_OUT, T], FP8, kind="ExternalOutput").ap()

    with tile.TileContext(nc) as tc:
        with (
            tc.tile_pool(name="x", bufs=1) as x_pool,
            tc.tile_pool(name="w8", bufs=2) as w8_pool,
            tc.tile_pool(name="w32", bufs=QUAD) as w32_pool,
            tc.tile_pool(name="bias", bufs=1) as b_pool,
            tc.tile_pool(name="out", bufs=6) as out_pool,
            tc.tile_pool(name="psum", bufs=8, space="PSUM") as psum_pool,
        ):
            xk = [
                x_pool.tile([P, T], F32R, tag=f"x_{k}", name=f"x_{k}")
                for k in range(KT)
            ]
            w8_tiles = {}
            w32_tiles = {}

            def stage_w8(b):
                w8 = w8_pool.tile([P, KT, P], FP8, tag="w8", name=f"w8_{b}")
                nc.sync.dma_start(w8[:], w_d[:, b])
                w8_tiles[b] = w8

            def cast_w(b):
                w32 = w32_pool.tile([P, KT, P], F32R, tag="w32", name=f"w32_{b}")
                nc.vector.tensor_copy(w32[:], w8_tiles.pop(b)[:])
                w32_tiles[b] = w32

            # Sync queue: first W slabs and early odd x tiles interleaved so
            # neither the first casts nor the early k-tiles arrive late; the
            # even x tiles stream on gpsimd in parallel.
            stage_w8(0)
            nc.sync.dma_start(
                xk[1][:], x_d[P : 2 * P, :]
            )
            stage_w8(1)
            for k in range(0, KT, 2):  # even k: gpsimd queue
                nc.gpsimd.dma_start(xk[k][:], x_d[k * P : (k + 1) * P, :])
            nc.sync.dma_start(xk[3][:], x_d[3 * P : 4 * P, :])
            bias_sb = b_pool.tile([P, MT], F32, tag="bias")
            nc.sync.dma_start(bias_sb[:], b_d[:, :])
            stage_w8(2)
            nc.sync.dma_start(xk[5][:], x_d[5 * P : 6 * P, :])
            stage_w8(3)
            for k in range(7, KT, 2):  # remaining odd k
                nc.sync.dma_start(xk[k][:], x_d[k * P : (k + 1) * P, :])

            cast_w(0)
            cast_w(1)
            cast_w(2)
            cast_w(3)

            nsls = [slice(n * TB, (n + 1) * TB) for n in range(NB)]

            def mm(psums, b, k, start, stop):
                for n in range(NB):
                    nc.tensor.matmul(
                        psums[(b, n)][:],
                        w32_tiles[b][:, k, :],
                        xk[k][:, nsls[n]],
                        start=start,
                        stop=stop,
                    )

            def epilogue(b, psums):
                for n in range(NB):
                    osb = out_pool.tile([P, TB], FP8, tag="osb",
                                        name=f"osb_{b}_{n}")
                    nc.scalar.activation(
                        osb[:], psums[(b, n)][:], SIGN,
                        bias=bias_sb[:, b : b + 1],
                    )
                    nc.scalar.dma_start(out_d[b * P : (b + 1) * P, nsls[n]], osb[:])

            # Quad phase: blocks 0..3 k-major across all 8 PSUM banks.
            qps = {
                (b, n): psum_pool.tile([P, TB], F32, tag="psum", name=f"ps_{b}_{n}")
                for b in range(QUAD)
                for n in range(NB)
            }
            for k in range(KT):
                mm(qps, 0, k, start=(k == 0), stop=(k == KT - 1))
                mm(qps, 1, k, start=(k == 0), stop=(k == KT - 1))
                if k >= KSTAG:
                    mm(qps, 2, k, start=(k == KSTAG), stop=False)
                    mm(qps, 3, k, start=(k == KSTAG), stop=False)
            epilogue(0, qps)
            epilogue(1, qps)
            for k in range(KSTAG):  # blocks 2,3 sweep their deferred head
                mm(qps, 2, k, start=False, stop=(k == KSTAG - 1))
                mm(qps, 3, k, start=False, stop=(k == KSTAG - 1))
            epilogue(2, qps)
            epilogue(3, qps)

            # Steady state: one block at a time, PSUM 4-deep pipelined.
            for b in range(QUAD, MT):
                stage_w8(b)
                cast_w(b)
                psums = {
                    (b, n): psum_pool.tile([P, TB], F32, tag="psum",
                                           name=f"ps_{b}_{n}")
                    for n in range(NB)
                }
                for k in range(KT):
                    mm(psums, b, k, start=(k == 0), stop=(k == KT - 1))
                epilogue(b, psums)
    nc.compile()
    _nc_cache = nc
    return nc


def prep_in_maps(x, weight, bias):
    """Host-side layout prep: transposes, fp8 sign-weight slabs, token shards."""
    x = np.asarray(x, dtype=np.float32)
    weight = np.asarray(weight, dtype=np.float32)
    bias = np.asarray(bias, dtype=np.float32)

    x_t = np.ascontiguousarray(x.T)  # [D_IN, N_TOK]
    # w8_slab[p, b, kt, m] = sign(W)[b*128+m, kt*128+p]
    w8 = np.sign(weight).astype(E4M3).reshape(MT, P, KT, P)
    w8_slab = np.ascontiguousarray(w8.transpose(3, 0, 2, 1))
    bias_t = np.ascontiguousarray(bias.reshape(MT, P).T)  # [P, MT]

    in_maps = []
    for c in range(N_CORES):
        sl = slice(c * T, (c + 1) * T)
        in_maps.append(
            {
                "x_t": np.ascontiguousarray(x_t[:, sl]),
                "w8_slab": w8_slab,
                "bias_t": bias_t,
            }
        )
    return in_maps


def run(x, weight, bias, **spmd_kwargs):
    """Run on the 8 cores; returns (full_output, BassKernelResults)."""
    nc = build()
    in_maps = prep_in_maps(x, weight, bias)
    res = run_bass_kernel_spmd(nc, in_maps, core_ids=list(range(N_CORES)), **spmd_kwargs)
    out = np.empty((N_TOK, D_OUT), dtype=np.float32)
    for c in range(N_CORES):
        out[c * T : (c + 1) * T, :] = res.results[c]["out_t"].astype(np.float32).T
    return out, res


def kernel(x, weight, bias):
    out, _ = run(x, weight, bias)
    return out


# revision 7
# speedup vs baseline: 1.4173x; 1.0004x over previous
"""Trainium2 Bass kernel for nn_BinaryLinear: out = sign(x @ sign(W).T + bias).

Strategy
--------
Data-parallel over the 8192-token dim: each of the 8 cores gets 1024 tokens
and the full weight matrix.

On-chip compute (per core) is the NT GEMM z.T = sign(W) @ x.T on the
TensorEngine with the contraction (in_features) on the partition dim:

  psum[outf, tok] = sum_k w[k, outf] * x[k, tok]

Precision/speed: the moving operand x is kept in float32r ("fp32 reduced"):
the PE reads 4-byte fp32 and rounds to 11 explicit mantissa bits, RNE
(probed on HW with one-hot weights). HW-measured, an fp32r matmul with
free dim 512 issues at ~224ns — the same ~1 row/cycle rate as fp16/bf16/
fp8 (every matmul on this part streams one moving row per cycle at 2.4GHz
regardless of dtype; fp8 DoubleRow doubles contraction per instruction but
fp8's 3-bit mantissa then needs 3 planes = more instructions than one
12-bit fp32r pass). A single fp32r pass is the instruction-count optimum:

  32 k-tiles x 32 outf-tiles x 2 token-blocks = 2048 matmuls x ~224ns
  ~= 460us/core, vs 48 instr/block (~660us) for any fp16+fp8 / 3xfp8
  scheme. Weights +-1 are exact in any dtype; measured end-to-end
  rel_err 1.1e-2 vs the 2e-2 budget.

Layout/DMA: every transfer is contiguous per partition (strided-gather
DMAs on this part are descriptor-bound at ~4.6ns per element):
  - W ships as e4m3 +-1 pre-arranged host-side into per-block slabs
    w8[p, b, kt, m] so block b stages with ONE 512KB DMA (4KB/partition),
    then the otherwise-idle VectorE upconverts it to an fp32r SBUF tile
    (fp32r weights must go through the self-loading matmul; standalone
    ldweights is broken for 4-byte dtypes).
  - bias ships pre-transposed [128, MT] (the naive "(mo p) -> p mo"
    rearrange DMA is 4096 4-byte descriptors = 21.7us of queue time).
  - output is written as e4m3 (sign is +-1, exact) z.T [out_f, tok] and
    untransposed/upcast on the host.

Schedule: x k-tiles stream on two queues (even k on gpsimd, odd k
interleaved with the first W slabs on sync). The first four blocks run
k-major interleaved across all 8 PSUM banks — blocks 2,3 join at k>=8 and
sweep their k<8 tail afterwards — so the PE tracks the incoming x stream
instead of stalling on the last k-tile of block 0. Remaining blocks run
sequentially (2 banks each, 4-deep pipelined). The epilogue (bias-add +
sign + PSUM->SBUF in one ScalarE activation; bias is per-partition in the
z.T layout) plus output DMAs live on the scalar queue so they never queue
behind the W/x streams.
"""

import numpy as np

import concourse.tile as tile
import concourse.mybir as mybir
from concourse import bacc
from concourse.bass_utils import run_bass_kernel_spmd

N_CORES = 8
N_TOK = 8192
D_IN = 4096
D_OUT = 4096
P = 128
T = N_TOK // N_CORES  # 1024 tokens per core
KT = D_IN // P  # 32 contraction tiles
MT = D_OUT // P  # 32 out-feature tiles (= W blocks)
TB = 512  # token block (one PSUM bank of fp32)
NB = T // TB  # 2 token blocks per core
QUAD = 4  # leading blocks run k-major interleaved (8 PSUM banks)
JOIN_K = {0: 0, 1: 0, 2: 6, 3: 8}  # staggered joins (w32 casts land late)

F32 = mybir.dt.float32
F32R = mybir.dt.float32r
FP8 = mybir.dt.float8e4
SIGN = mybir.ActivationFunctionType.Sign
E4M3 = mybir.dt.np(FP8)

_nc_cache = None


def build():
    """Build + compile the per-core Bass/Tile module (SPMD: same on all cores)."""
    global _nc_cache
    if _nc_cache is not None:
        return _nc_cache
    nc = bacc.Bacc("TRN2", target_bir_lowering=False, debug=False, num_devices=N_CORES)
    x_d = nc.dram_tensor("x_t", [D_IN, T], F32R, kind="ExternalInput").ap()
    # per-block weight slabs: w8[p, b, kt, m] = sign(W)[b*128+m, kt*128+p]
    w_d = nc.dram_tensor("w8_slab", [P, MT, KT, P], FP8, kind="ExternalInput").ap()
    b_d = nc.dram_tensor("bias_t", [P, MT], F32, kind="ExternalInput").ap()
    out_d = nc.dram_tensor("out_t", [D_OUT, T], FP8, kind="ExternalOutput").ap()

    with tile.TileContext(nc) as tc:
        with (
            tc.tile_pool(name="x", bufs=1) as x_pool,
            tc.tile_pool(name="w8", bufs=2) as w8_pool,
            tc.tile_pool(name="w32", bufs=QUAD) as w32_pool,
            tc.tile_pool(name="bias", bufs=1) as b_pool,
            tc.tile_pool(name="out", bufs=6) as out_pool,
            tc.tile_pool(name="psum", bufs=8, space="PSUM") as psum_pool,
        ):
            xk = [
                x_pool.tile([P, T], F32R, tag=f"x_{k}", name=f"x_{k}")
                for k in range(KT)
            ]
            w8_tiles = {}
            w32_tiles = {}

            def stage_w8(b):
                w8 = w8_pool.tile([P, KT, P], FP8, tag="w8", name=f"w8_{b}")
                nc.sync.dma_start(w8[:], w_d[:, b])
                w8_tiles[b] = w8

            def cast_w(b):
                # two k-half casts so early matmuls unblock after half a slab
                w32 = w32_pool.tile([P, KT, P], F32R, tag="w32", name=f"w32_{b}")
                w8 = w8_tiles.pop(b)
                h = KT // 2
                nc.vector.tensor_copy(w32[:, :h, :], w8[:, :h, :])
                nc.vector.tensor_copy(w32[:, h:, :], w8[:, h:, :])
                w32_tiles[b] = w32

            # Sync queue: first W slabs and early odd x tiles interleaved so
            # neither the first casts nor the early k-tiles arrive late; the
            # even x tiles stream on gpsimd in parallel.
            stage_w8(0)
            nc.sync.dma_start(xk[1][:], x_d[P : 2 * P, :])
            stage_w8(1)
            for k in range(0, KT, 2):  # even k: gpsimd queue
                nc.gpsimd.dma_start(xk[k][:], x_d[k * P : (k + 1) * P, :])
            nc.sync.dma_start(xk[3][:], x_d[3 * P : 4 * P, :])
            stage_w8(2)
            nc.sync.dma_start(xk[5][:], x_d[5 * P : 6 * P, :])
            stage_w8(3)
            bias_sb = b_pool.tile([P, MT], F32, tag="bias")
            nc.sync.dma_start(bias_sb[:], b_d[:, :])
            for k in range(7, KT, 2):  # remaining odd k
                nc.sync.dma_start(xk[k][:], x_d[k * P : (k + 1) * P, :])

            cast_w(0)
            cast_w(1)
            cast_w(2)
            cast_w(3)

            nsls = [slice(n * TB, (n + 1) * TB) for n in range(NB)]

            def mm(psums, b, k, start, stop):
                for n in range(NB):
                    nc.tensor.matmul(
                        psums[(b, n)][:],
                        w32_tiles[b][:, k, :],
                        xk[k][:, nsls[n]],
                        start=start,
                        stop=stop,
                    )

            def epilogue(b, psums):
                for n in range(NB):
                    osb = out_pool.tile([P, TB], FP8, tag="osb",
                                        name=f"osb_{b}_{n}")
                    nc.scalar.activation(
                        osb[:], psums[(b, n)][:], SIGN,
                        bias=bias_sb[:, b : b + 1],
                    )
                    nc.scalar.dma_start(out_d[b * P : (b + 1) * P, nsls[n]], osb[:])

            # Quad phase: blocks 0..3 k-major across all 8 PSUM banks.
            qps = {
                (b, n): psum_pool.tile([P, TB], F32, tag="psum", name=f"ps_{b}_{n}")
                for b in range(QUAD)
                for n in range(NB)
            }
            for k in range(KT):
                for b in range(QUAD):
                    if k >= JOIN_K[b]:
                        mm(qps, b, k, start=(k == JOIN_K[b]),
                           stop=(k == KT - 1 and JOIN_K[b] == 0))
            epilogue(0, qps)
            epilogue(1, qps)
            for b in (2, 3):  # sweep the deferred heads
                for k in range(JOIN_K[b]):
                    mm(qps, b, k, start=False, stop=(k == JOIN_K[b] - 1))
                epilogue(b, qps)

            # Steady state: one block at a time, PSUM 4-deep pipelined.
            for b in range(QUAD, MT):
                stage_w8(b)
                cast_w(b)
                psums = {
                    (b, n): psum_pool.tile([P, TB], F32, tag="psum",
                                           name=f"ps_{b}_{n}")
                    for n in range(NB)
                }
                for k in range(KT):
                    mm(psums, b, k, start=(k == 0), stop=(k == KT - 1))
                epilogue(b, psums)
    nc.compile()
    _nc_cache = nc
    return nc


def prep_in_maps(x, weight, bias):
    """Host-side layout prep: transposes, fp8 sign-weight slabs, token shards."""
    x = np.asarray(x, dtype=np.float32)
    weight = np.asarray(weight, dtype=np.float32)
    bias = np.asarray(bias, dtype=np.float32)

    x_t = np.ascontiguousarray(x.T)  # [D_IN, N_TOK]
    # w8_slab[p, b, kt, m] = sign(W)[b*128+m, kt*128+p]
    w8 = np.sign(weight).astype(E4M3).reshape(MT, P, KT, P)
    w8_slab = np.ascontiguousarray(w8.transpose(3, 0, 2, 1))
    bias_t = np.ascontiguousarray(bias.reshape(MT, P).T)  # [P, MT]

    in_maps = []
    for c in range(N_CORES):
        sl = slice(c * T, (c + 1) * T)
        in_maps.append(
            {
                "x_t": np.ascontiguousarray(x_t[:, sl]),
                "w8_slab": w8_slab,
                "bias_t": bias_t,
            }
        )
    return in_maps


def run(x, weight, bias, **spmd_kwargs):
    """Run on the 8 cores; returns (full_output, BassKernelResults)."""
    nc = build()
    in_maps = prep_in_maps(x, weight, bias)
    res = run_bass_kernel_spmd(nc, in_maps, core_ids=list(range(N_CORES)), **spmd_kwargs)
    out = np.empty((N_TOK, D_OUT), dtype=np.float32)
    for c in range(N_CORES):
        out[c * T : (c + 1) * T, :] = res.results[c]["out_t"].astype(np.float32).T
    return out, res


def kernel(x, weight, bias):
    out, _ = run(x, weight, bias)
    return out


# revision 9
# speedup vs baseline: 1.4319x; 1.0103x over previous
"""Trainium2 Bass kernel for nn_BinaryLinear: out = sign(x @ sign(W).T + bias).

Strategy
--------
Data-parallel over the 8192-token dim: each of the 8 cores gets 1024 tokens
and the full weight matrix.

On-chip compute (per core) is the NT GEMM z.T = sign(W) @ x.T on the
TensorEngine with the contraction (in_features) on the partition dim:

  psum[outf, tok] = sum_k w[k, outf] * x[k, tok]

Precision/speed: the moving operand x is kept in float32r ("fp32 reduced"):
the PE reads 4-byte fp32 and rounds to 11 explicit mantissa bits, RNE
(probed on HW with one-hot weights). HW-measured, an fp32r matmul with
free dim 512 issues at ~224ns — the same ~1 row/cycle rate as fp16/bf16/
fp8 (every matmul on this part streams one moving row per cycle at 2.4GHz
regardless of dtype; fp8 DoubleRow doubles contraction per instruction but
fp8's 3-bit mantissa then needs 3 planes = more instructions than one
12-bit fp32r pass). A single fp32r pass is the instruction-count optimum:

  32 k-tiles x 32 outf-tiles x 2 token-blocks = 2048 matmuls x ~224ns
  ~= 460us/core, vs 48 instr/block (~660us) for any fp16+fp8 / 3xfp8
  scheme. Weights +-1 are exact in any dtype; measured end-to-end
  rel_err 1.1e-2 vs the 2e-2 budget.

Layout/DMA: every transfer is contiguous per partition (strided-gather
DMAs on this part are descriptor-bound at ~4.6ns per element):
  - W ships as e4m3 +-1 pre-arranged host-side into per-block slabs
    w8[p, b, kt, m] so block b stages with ONE 512KB DMA (4KB/partition),
    then the otherwise-idle VectorE upconverts it to an fp32r SBUF tile
    (fp32r weights must go through the self-loading matmul; standalone
    ldweights is broken for 4-byte dtypes).
  - bias ships pre-transposed [128, MT] (the naive "(mo p) -> p mo"
    rearrange DMA is 4096 4-byte descriptors = 21.7us of queue time).
  - output is written as e4m3 (sign is +-1, exact) z.T [out_f, tok] and
    untransposed/upcast on the host.

Schedule: x k-tiles stream on two queues (even k on gpsimd, odd k
interleaved with the first W slabs on sync). The first four blocks run
k-major interleaved across all 8 PSUM banks — blocks 2,3 join at k>=8 and
sweep their k<8 tail afterwards — so the PE tracks the incoming x stream
instead of stalling on the last k-tile of block 0. Remaining blocks run
sequentially (2 banks each, 4-deep pipelined). The epilogue (bias-add +
sign + PSUM->SBUF in one ScalarE activation; bias is per-partition in the
z.T layout) plus output DMAs live on the scalar queue so they never queue
behind the W/x streams.
"""

import numpy as np

import concourse.tile as tile
import concourse.mybir as mybir
from concourse import bacc
from concourse.bass_utils import run_bass_kernel_spmd

N_CORES = 8
N_TOK = 8192
D_IN = 4096
D_OUT = 4096
P = 128
T = N_TOK // N_CORES  # 1024 tokens per core
KT = D_IN // P  # 32 contraction tiles
MT = D_OUT // P  # 32 out-feature tiles (= W blocks)
TB = 512  # token block (one PSUM bank of fp32)
NB = T // TB  # 2 token blocks per core
QUAD = 4  # leading blocks run k-major interleaved (8 PSUM banks)
JOIN_K = {0: 0, 1: 0, 2: 6, 3: 8}  # staggered joins (w32 casts land late)

F32 = mybir.dt.float32
F32R = mybir.dt.float32r
FP8 = mybir.dt.float8e4
SIGN = mybir.ActivationFunctionType.Sign
E4M3 = mybir.dt.np(FP8)

_nc_cache = None


def build():
    """Build + compile the per-core Bass/Tile module (SPMD: same on all cores)."""
    global _nc_cache
    if _nc_cache is not None:
        return _nc_cache
    nc = bacc.Bacc("TRN2", target_bir_lowering=False, debug=False, num_devices=N_CORES)
    x_d = nc.dram_tensor("x_t", [D_IN, T], F32R, kind="ExternalInput").ap()
    # per-block weight slabs: w8[p, b, kt, m] = sign(W)[b*128+m, kt*128+p]
    w_d = nc.dram_tensor("w8_slab", [P, MT, KT, P], FP8, kind="ExternalInput").ap()
    b_d = nc.dram_tensor("bias_t", [P, MT], F32, kind="ExternalInput").ap()
    out_d = nc.dram_tensor("out_t", [D_OUT, T], FP8, kind="ExternalOutput").ap()

    with tile.TileContext(nc) as tc:
        with (
            tc.tile_pool(name="x", bufs=1) as x_pool,
            tc.tile_pool(name="w8", bufs=2) as w8_pool,
            tc.tile_pool(name="w32", bufs=QUAD) as w32_pool,
            tc.tile_pool(name="bias", bufs=1) as b_pool,
            tc.tile_pool(name="out", bufs=6) as out_pool,
            tc.tile_pool(name="psum", bufs=8, space="PSUM") as psum_pool,
        ):
            xk = [
                x_pool.tile([P, T], F32R, tag=f"x_{k}", name=f"x_{k}")
                for k in range(KT)
            ]
            w8_tiles = {}
            w32_tiles = {}

            def stage_w8(b):
                w8 = w8_pool.tile([P, KT, P], FP8, tag="w8", name=f"w8_{b}")
                nc.sync.dma_start(w8[:], w_d[:, b])
                w8_tiles[b] = w8

            def cast_w(b):
                # two k-half casts so early matmuls unblock after half a slab
                w32 = w32_pool.tile([P, KT, P], F32R, tag="w32", name=f"w32_{b}")
                w8 = w8_tiles.pop(b)
                h = KT // 2
                nc.vector.tensor_copy(w32[:, :h, :], w8[:, :h, :])
                nc.vector.tensor_copy(w32[:, h:, :], w8[:, h:, :])
                w32_tiles[b] = w32

            # Sync queue: first W slabs and early odd x tiles interleaved so
            # neither the first casts nor the early k-tiles arrive late; the
            # even x tiles stream on gpsimd in parallel.
            def dma_x(queue, k):
                # two token-half transfers: n=0 matmuls unblock at half-tile
                for n in range(NB):
                    sl = slice(n * TB, (n + 1) * TB)
                    queue.dma_start(xk[k][:, sl], x_d[k * P : (k + 1) * P, sl])

            stage_w8(0)
            dma_x(nc.sync, 1)
            stage_w8(1)
            for k in range(0, KT, 2):  # even k: gpsimd queue
                dma_x(nc.gpsimd, k)
            dma_x(nc.sync, 3)
            stage_w8(2)
            dma_x(nc.sync, 5)
            stage_w8(3)
            bias_sb = b_pool.tile([P, MT], F32, tag="bias")
            nc.sync.dma_start(bias_sb[:], b_d[:, :])
            for k in range(7, KT, 2):  # remaining odd k
                dma_x(nc.sync, k)

            cast_w(0)
            cast_w(1)
            cast_w(2)
            cast_w(3)

            nsls = [slice(n * TB, (n + 1) * TB) for n in range(NB)]

            def mm(psums, b, k, start, stop):
                for n in range(NB):
                    nc.tensor.matmul(
                        psums[(b, n)][:],
                        w32_tiles[b][:, k, :],
                        xk[k][:, nsls[n]],
                        start=start,
                        stop=stop,
                    )

            def epilogue(b, psums):
                for n in range(NB):
                    osb = out_pool.tile([P, TB], FP8, tag="osb",
                                        name=f"osb_{b}_{n}")
                    nc.scalar.activation(
                        osb[:], psums[(b, n)][:], SIGN,
                        bias=bias_sb[:, b : b + 1],
                    )
                    nc.scalar.dma_start(out_d[b * P : (b + 1) * P, nsls[n]], osb[:])

            # Quad phase: blocks 0..3 k-major across all 8 PSUM banks.
            qps = {
                (b, n): psum_pool.tile([P, TB], F32, tag="psum", name=f"ps_{b}_{n}")
                for b in range(QUAD)
                for n in range(NB)
            }
            for k in range(KT):
                for n in range(NB):  # n-major: n=0 runs on half-arrived tiles
                    for b in range(QUAD):
                        if k >= JOIN_K[b]:
                            nc.tensor.matmul(
                                qps[(b, n)][:],
                                w32_tiles[b][:, k, :],
                                xk[k][:, nsls[n]],
                                start=(k == JOIN_K[b]),
                                stop=(k == KT - 1 and JOIN_K[b] == 0),
                            )
            epilogue(0, qps)
            epilogue(1, qps)
            for b in (2, 3):  # sweep the deferred heads
                for k in range(JOIN_K[b]):
                    mm(qps, b, k, start=False, stop=(k == JOIN_K[b] - 1))
                epilogue(b, qps)

            # Steady state: one block at a time, PSUM 4-deep pipelined.
            for b in range(QUAD, MT):
                stage_w8(b)
                cast_w(b)
                psums = {
                    (b, n): psum_pool.tile([P, TB], F32, tag="psum",
                                           name=f"ps_{b}_{n}")
                    for n in range(NB)
                }
                for k in range(KT):
                    mm(psums, b, k, start=(k == 0), stop=(k == KT - 1))
                epilogue(b, psums)
    nc.compile()
    _nc_cache = nc
    return nc


def prep_in_maps(x, weight, bias):
    """Host-side layout prep: transposes, fp8 sign-weight slabs, token shards."""
    x = np.asarray(x, dtype=np.float32)
    weight = np.asarray(weight, dtype=np.float32)
    bias = np.asarray(bias, dtype=np.float32)

    x_t = np.ascontiguousarray(x.T)  # [D_IN, N_TOK]
    # w8_slab[p, b, kt, m] = sign(W)[b*128+m, kt*128+p]
    w8 = np.sign(weight).astype(E4M3).reshape(MT, P, KT, P)
    w8_slab = np.ascontiguousarray(w8.transpose(3, 0, 2, 1))
    bias_t = np.ascontiguousarray(bias.reshape(MT, P).T)  # [P, MT]

    in_maps = []
    for c in range(N_CORES):
        sl = slice(c * T, (c + 1) * T)
        in_maps.append(
            {
                "x_t": np.ascontiguousarray(x_t[:, sl]),
                "w8_slab": w8_slab,
                "bias_t": bias_t,
            }
        )
    return in_maps


def run(x, weight, bias, **spmd_kwargs):
    """Run on the 8 cores; returns (full_output, BassKernelResults)."""
    nc = build()
    in_maps = prep_in_maps(x, weight, bias)
    res = run_bass_kernel_spmd(nc, in_maps, core_ids=list(range(N_CORES)), **spmd_kwargs)
    out = np.empty((N_TOK, D_OUT), dtype=np.float32)
    for c in range(N_CORES):
        out[c * T : (c + 1) * T, :] = res.results[c]["out_t"].astype(np.float32).T
    return out, res


def kernel(x, weight, bias):
    out, _ = run(x, weight, bias)
    return out


# revision 12
# speedup vs baseline: 1.4337x; 1.0012x over previous
"""Trainium2 Bass kernel for nn_BinaryLinear: out = sign(x @ sign(W).T + bias).

Strategy
--------
Data-parallel over the 8192-token dim: each of the 8 cores gets 1024 tokens
and the full weight matrix.

On-chip compute (per core) is the NT GEMM z.T = sign(W) @ x.T on the
TensorEngine with the contraction (in_features) on the partition dim:

  psum[outf, tok] = sum_k w[k, outf] * x[k, tok]

Precision/speed: the moving operand x is kept in float32r ("fp32 reduced"):
the PE reads 4-byte fp32 and rounds to 11 explicit mantissa bits, RNE
(probed on HW with one-hot weights). HW-measured, an fp32r matmul with
free dim 512 issues at ~224ns — the same ~1 row/cycle rate as fp16/bf16/
fp8 (every matmul on this part streams one moving row per cycle at 2.4GHz
regardless of dtype; fp8 DoubleRow doubles contraction per instruction but
fp8's 3-bit mantissa then needs 3 planes = more instructions than one
12-bit fp32r pass). A single fp32r pass is the instruction-count optimum:

  32 k-tiles x 32 outf-tiles x 2 token-blocks = 2048 matmuls x ~224ns
  ~= 460us/core, vs 48 instr/block (~660us) for any fp16+fp8 / 3xfp8
  scheme. Weights +-1 are exact in any dtype; measured end-to-end
  rel_err 1.1e-2 vs the 2e-2 budget.

Layout/DMA: every transfer is contiguous per partition (strided-gather
DMAs on this part are descriptor-bound at ~4.6ns per element):
  - W ships as e4m3 +-1 pre-arranged host-side into per-block slabs
    w8[p, b, kt, m] so block b stages with ONE 512KB DMA (4KB/partition),
    then the otherwise-idle VectorE upconverts it to an fp32r SBUF tile
    (fp32r weights must go through the self-loading matmul; standalone
    ldweights is broken for 4-byte dtypes).
  - bias ships pre-transposed [128, MT] (the naive "(mo p) -> p mo"
    rearrange DMA is 4096 4-byte descriptors = 21.7us of queue time).
  - output is written as e4m3 (sign is +-1, exact) z.T [out_f, tok] and
    untransposed/upcast on the host.

Schedule: x k-tiles stream on two queues (even k on gpsimd, odd k
interleaved with the first W slabs on sync). The first four blocks run
k-major interleaved across all 8 PSUM banks — blocks 2,3 join at k>=8 and
sweep their k<8 tail afterwards — so the PE tracks the incoming x stream
instead of stalling on the last k-tile of block 0. Remaining blocks run
sequentially (2 banks each, 4-deep pipelined). The epilogue (bias-add +
sign + PSUM->SBUF in one ScalarE activation; bias is per-partition in the
z.T layout) plus output DMAs live on the scalar queue so they never queue
behind the W/x streams.
"""

import numpy as np

import concourse.tile as tile
import concourse.mybir as mybir
from concourse import bacc
from concourse.bass_utils import run_bass_kernel_spmd

N_CORES = 8
N_TOK = 8192
D_IN = 4096
D_OUT = 4096
P = 128
T = N_TOK // N_CORES  # 1024 tokens per core
KT = D_IN // P  # 32 contraction tiles
MT = D_OUT // P  # 32 out-feature tiles (= W blocks)
TB = 512  # token block (one PSUM bank of fp32)
NB = T // TB  # 2 token blocks per core
QUAD = 4  # leading blocks run k-major interleaved (8 PSUM banks)
JOIN_K = {0: 0, 1: 0, 2: 4, 3: 6}  # staggered joins (w32 casts land late)

F32 = mybir.dt.float32
F32R = mybir.dt.float32r
FP8 = mybir.dt.float8e4
SIGN = mybir.ActivationFunctionType.Sign
E4M3 = mybir.dt.np(FP8)

_nc_cache = None


def build():
    """Build + compile the per-core Bass/Tile module (SPMD: same on all cores)."""
    global _nc_cache
    if _nc_cache is not None:
        return _nc_cache
    nc = bacc.Bacc("TRN2", target_bir_lowering=False, debug=False, num_devices=N_CORES)
    x_d = nc.dram_tensor("x_t", [D_IN, T], F32R, kind="ExternalInput").ap()
    # per-block weight slabs: w8[p, b, kt, m] = sign(W)[b*128+m, kt*128+p]
    w_d = nc.dram_tensor("w8_slab", [P, MT, KT, P], FP8, kind="ExternalInput").ap()
    b_d = nc.dram_tensor("bias_t", [P, MT], F32, kind="ExternalInput").ap()
    out_d = nc.dram_tensor("out_t", [D_OUT, T], FP8, kind="ExternalOutput").ap()

    with tile.TileContext(nc) as tc:
        with (
            tc.tile_pool(name="x", bufs=1) as x_pool,
            tc.tile_pool(name="w8", bufs=2) as w8_pool,
            tc.tile_pool(name="w32", bufs=QUAD) as w32_pool,
            tc.tile_pool(name="bias", bufs=1) as b_pool,
            tc.tile_pool(name="out", bufs=6) as out_pool,
            tc.tile_pool(name="psum", bufs=8, space="PSUM") as psum_pool,
        ):
            xk = [
                x_pool.tile([P, T], F32R, tag=f"x_{k}", name=f"x_{k}")
                for k in range(KT)
            ]
            w8_tiles = {}
            w32_tiles = {}

            def stage_w8(b):
                w8 = w8_pool.tile([P, KT, P], FP8, tag="w8", name=f"w8_{b}")
                nc.sync.dma_start(w8[:], w_d[:, b])
                w8_tiles[b] = w8

            def cast_w(b, splits=2):
                # k-split casts so early matmuls unblock after a partial slab
                w32 = w32_pool.tile([P, KT, P], F32R, tag="w32", name=f"w32_{b}")
                w8 = w8_tiles.pop(b)
                h = KT // splits
                for s in range(splits):
                    nc.vector.tensor_copy(
                        w32[:, s * h : (s + 1) * h, :], w8[:, s * h : (s + 1) * h, :]
                    )
                w32_tiles[b] = w32

            # Sync queue: first W slabs and early odd x tiles interleaved so
            # neither the first casts nor the early k-tiles arrive late; the
            # even x tiles stream on gpsimd in parallel.
            def dma_x(queue, k):
                # two token-half transfers: n=0 matmuls unblock at half-tile
                for n in range(NB):
                    sl = slice(n * TB, (n + 1) * TB)
                    queue.dma_start(xk[k][:, sl], x_d[k * P : (k + 1) * P, sl])

            stage_w8(0)
            dma_x(nc.sync, 1)
            stage_w8(1)
            for k in range(0, KT, 2):  # even k: gpsimd queue
                dma_x(nc.gpsimd, k)
            dma_x(nc.sync, 3)
            stage_w8(2)
            dma_x(nc.sync, 5)
            stage_w8(3)
            bias_sb = b_pool.tile([P, MT], F32, tag="bias")
            nc.sync.dma_start(bias_sb[:], b_d[:, :])
            for k in range(7, KT, 2):  # remaining odd k
                dma_x(nc.sync, k)

            cast_w(0, splits=4)
            cast_w(1, splits=2)
            cast_w(2, splits=2)
            cast_w(3, splits=2)

            nsls = [slice(n * TB, (n + 1) * TB) for n in range(NB)]

            def mm(psums, b, k, start, stop):
                for n in range(NB):
                    nc.tensor.matmul(
                        psums[(b, n)][:],
                        w32_tiles[b][:, k, :],
                        xk[k][:, nsls[n]],
                        start=start,
                        stop=stop,
                    )

            def epilogue(b, psums):
                for n in range(NB):
                    osb = out_pool.tile([P, TB], FP8, tag="osb",
                                        name=f"osb_{b}_{n}")
                    nc.scalar.activation(
                        osb[:], psums[(b, n)][:], SIGN,
                        bias=bias_sb[:, b : b + 1],
                    )
                    nc.scalar.dma_start(out_d[b * P : (b + 1) * P, nsls[n]], osb[:])

            # Quad phase: blocks 0..3 k-major across all 8 PSUM banks.
            qps = {
                (b, n): psum_pool.tile([P, TB], F32, tag="psum", name=f"ps_{b}_{n}")
                for b in range(QUAD)
                for n in range(NB)
            }
            for k in range(KT):
                for n in range(NB):  # n-major: n=0 runs on half-arrived tiles
                    for b in range(QUAD):
                        if k >= JOIN_K[b]:
                            nc.tensor.matmul(
                                qps[(b, n)][:],
                                w32_tiles[b][:, k, :],
                                xk[k][:, nsls[n]],
                                start=(k == JOIN_K[b]),
                                stop=(k == KT - 1 and JOIN_K[b] == 0),
                            )
            epilogue(0, qps)
            epilogue(1, qps)
            for b in (2, 3):  # sweep the deferred heads
                for k in range(JOIN_K[b]):
                    mm(qps, b, k, start=False, stop=(k == JOIN_K[b] - 1))
                epilogue(b, qps)

            # Steady state: one block at a time, PSUM 4-deep pipelined.
            for b in range(QUAD, MT):
                stage_w8(b)
                cast_w(b)
                psums = {
                    (b, n): psum_pool.tile([P, TB], F32, tag="psum",
                                           name=f"ps_{b}_{n}")
                    for n in range(NB)
                }
                for k in range(KT):
                    mm(psums, b, k, start=(k == 0), stop=(k == KT - 1))
                epilogue(b, psums)
    nc.compile()
    _nc_cache = nc
    return nc


def prep_in_maps(x, weight, bias):
    """Host-side layout prep: transposes, fp8 sign-weight slabs, token shards."""
    x = np.asarray(x, dtype=np.float32)
    weight = np.asarray(weight, dtype=np.float32)
    bias = np.asarray(bias, dtype=np.float32)

    x_t = np.ascontiguousarray(x.T)  # [D_IN, N_TOK]
    # w8_slab[p, b, kt, m] = sign(W)[b*128+m, kt*128+p]
    w8 = np.sign(weight).astype(E4M3).reshape(MT, P, KT, P)
    w8_slab = np.ascontiguousarray(w8.transpose(3, 0, 2, 1))
    bias_t = np.ascontiguousarray(bias.reshape(MT, P).T)  # [P, MT]

    in_maps = []
    for c in range(N_CORES):
        sl = slice(c * T, (c + 1) * T)
        in_maps.append(
            {
                "x_t": np.ascontiguousarray(x_t[:, sl]),
                "w8_slab": w8_slab,
                "bias_t": bias_t,
            }
        )
    return in_maps


def run(x, weight, bias, **spmd_kwargs):
    """Run on the 8 cores; returns (full_output, BassKernelResults)."""
    nc = build()
    in_maps = prep_in_maps(x, weight, bias)
    res = run_bass_kernel_spmd(nc, in_maps, core_ids=list(range(N_CORES)), **spmd_kwargs)
    out = np.empty((N_TOK, D_OUT), dtype=np.float32)
    for c in range(N_CORES):
        out[c * T : (c + 1) * T, :] = res.results[c]["out_t"].astype(np.float32).T
    return out, res


def kernel(x, weight, bias):
    out, _ = run(x, weight, bias)
    return out


# revision 13
# speedup vs baseline: 1.4396x; 1.0042x over previous
"""Trainium2 Bass kernel for nn_BinaryLinear: out = sign(x @ sign(W).T + bias).

Strategy
--------
Data-parallel over the 8192-token dim: each of the 8 cores gets 1024 tokens
and the full weight matrix.

On-chip compute (per core) is the NT GEMM z.T = sign(W) @ x.T on the
TensorEngine with the contraction (in_features) on the partition dim:

  psum[outf, tok] = sum_k w[k, outf] * x[k, tok]

Precision/speed: the moving operand x is kept in float32r ("fp32 reduced"):
the PE reads 4-byte fp32 and rounds to 11 explicit mantissa bits, RNE
(probed on HW with one-hot weights). HW-measured, an fp32r matmul with
free dim 512 issues at ~224ns — the same ~1 row/cycle rate as fp16/bf16/
fp8 (every matmul on this part streams one moving row per cycle at 2.4GHz
regardless of dtype; fp8 DoubleRow doubles contraction per instruction but
fp8's 3-bit mantissa then needs 3 planes = more instructions than one
12-bit fp32r pass). A single fp32r pass is the instruction-count optimum:

  32 k-tiles x 32 outf-tiles x 2 token-blocks = 2048 matmuls x ~224ns
  ~= 460us/core, vs 48 instr/block (~660us) for any fp16+fp8 / 3xfp8
  scheme. Weights +-1 are exact in any dtype; measured end-to-end
  rel_err 1.1e-2 vs the 2e-2 budget.

Layout/DMA: every transfer is contiguous per partition (strided-gather
DMAs on this part are descriptor-bound at ~4.6ns per element):
  - W ships as e4m3 +-1 pre-arranged host-side into per-block slabs
    w8[p, b, kt, m] so block b stages with ONE 512KB DMA (4KB/partition),
    then the otherwise-idle VectorE upconverts it to an fp32r SBUF tile
    (fp32r weights must go through the self-loading matmul; standalone
    ldweights is broken for 4-byte dtypes).
  - bias ships pre-transposed [128, MT] (the naive "(mo p) -> p mo"
    rearrange DMA is 4096 4-byte descriptors = 21.7us of queue time).
  - output is written as e4m3 (sign is +-1, exact) z.T [out_f, tok] and
    untransposed/upcast on the host.

Schedule: x k-tiles stream on two queues in token-halves (even k on
gpsimd, odd k interleaved with the first W slabs on sync; the n=0 matmuls
unblock on a half-arrived tile). The first four blocks run k-major
interleaved across all 8 PSUM banks — blocks 2,3 join at k>=JOIN_K and
sweep their deferred head afterwards — so the PE tracks the incoming x
stream instead of stalling on the last k-tile of block 0. Remaining
blocks run sequentially (2 banks each, 4-deep pipelined). The epilogue
(bias-add + sign + PSUM->SBUF in one ScalarE activation; bias is
per-partition in the z.T layout) plus output DMAs live on the scalar
queue so they never queue behind the W/x streams.

Measured: 495.4us HW exec (vs 710.2us for the previous fp16+fp8 hi/lo
kernel), rel_err 1.137e-2, 1085/33.5M sign flips — bit-identical to an
m11-RNE numpy simulation of the same inputs.
"""

import numpy as np

import concourse.tile as tile
import concourse.mybir as mybir
from concourse import bacc
from concourse.bass_utils import run_bass_kernel_spmd

N_CORES = 8
N_TOK = 8192
D_IN = 4096
D_OUT = 4096
P = 128
T = N_TOK // N_CORES  # 1024 tokens per core
KT = D_IN // P  # 32 contraction tiles
MT = D_OUT // P  # 32 out-feature tiles (= W blocks)
TB = 512  # token block (one PSUM bank of fp32)
NB = T // TB  # 2 token blocks per core
QUAD = 4  # leading blocks run k-major interleaved (8 PSUM banks)
JOIN_K = {0: 0, 1: 0, 2: 4, 3: 6}  # staggered joins (w32 casts land late)

F32 = mybir.dt.float32
F32R = mybir.dt.float32r
FP8 = mybir.dt.float8e4
SIGN = mybir.ActivationFunctionType.Sign
E4M3 = mybir.dt.np(FP8)

_nc_cache = None


def build():
    """Build + compile the per-core Bass/Tile module (SPMD: same on all cores)."""
    global _nc_cache
    if _nc_cache is not None:
        return _nc_cache
    nc = bacc.Bacc("TRN2", target_bir_lowering=False, debug=False, num_devices=N_CORES)
    x_d = nc.dram_tensor("x_t", [D_IN, T], F32R, kind="ExternalInput").ap()
    # per-block weight slabs: w8[p, b, kt, m] = sign(W)[b*128+m, kt*128+p]
    w_d = nc.dram_tensor("w8_slab", [P, MT, KT, P], FP8, kind="ExternalInput").ap()
    b_d = nc.dram_tensor("bias_t", [P, MT], F32, kind="ExternalInput").ap()
    out_d = nc.dram_tensor("out_t", [D_OUT, T], FP8, kind="ExternalOutput").ap()

    with tile.TileContext(nc) as tc:
        with (
            tc.tile_pool(name="x", bufs=1) as x_pool,
            tc.tile_pool(name="w8", bufs=2) as w8_pool,
            tc.tile_pool(name="w32", bufs=QUAD) as w32_pool,
            tc.tile_pool(name="bias", bufs=1) as b_pool,
            tc.tile_pool(name="out", bufs=6) as out_pool,
            tc.tile_pool(name="psum", bufs=8, space="PSUM") as psum_pool,
        ):
            xk = [
                x_pool.tile([P, T], F32R, tag=f"x_{k}", name=f"x_{k}")
                for k in range(KT)
            ]
            w8_tiles = {}
            w32_tiles = {}

            def stage_w8(b):
                w8 = w8_pool.tile([P, KT, P], FP8, tag="w8", name=f"w8_{b}")
                nc.sync.dma_start(w8[:], w_d[:, b])
                w8_tiles[b] = w8

            def cast_w(b, splits=2):
                # k-split casts so early matmuls unblock after a partial slab
                w32 = w32_pool.tile([P, KT, P], F32R, tag="w32", name=f"w32_{b}")
                w8 = w8_tiles.pop(b)
                h = KT // splits
                for s in range(splits):
                    nc.vector.tensor_copy(
                        w32[:, s * h : (s + 1) * h, :], w8[:, s * h : (s + 1) * h, :]
                    )
                w32_tiles[b] = w32

            # Sync queue: first W slabs and early odd x tiles interleaved so
            # neither the first casts nor the early k-tiles arrive late; the
            # even x tiles stream on gpsimd in parallel.
            def dma_x(queue, k):
                # two token-half transfers: n=0 matmuls unblock at half-tile
                for n in range(NB):
                    sl = slice(n * TB, (n + 1) * TB)
                    queue.dma_start(xk[k][:, sl], x_d[k * P : (k + 1) * P, sl])

            stage_w8(0)
            dma_x(nc.sync, 1)
            stage_w8(1)
            for k in range(0, KT, 2):  # even k: gpsimd queue
                dma_x(nc.gpsimd, k)
            dma_x(nc.sync, 3)
            stage_w8(2)
            dma_x(nc.sync, 5)
            stage_w8(3)
            bias_sb = b_pool.tile([P, MT], F32, tag="bias")
            nc.sync.dma_start(bias_sb[:], b_d[:, :])
            for k in range(7, KT, 2):  # remaining odd k
                dma_x(nc.sync, k)

            cast_w(0, splits=4)
            cast_w(1, splits=2)
            cast_w(2, splits=2)
            cast_w(3, splits=2)

            nsls = [slice(n * TB, (n + 1) * TB) for n in range(NB)]

            def mm(psums, b, k, start, stop):
                for n in range(NB):
                    nc.tensor.matmul(
                        psums[(b, n)][:],
                        w32_tiles[b][:, k, :],
                        xk[k][:, nsls[n]],
                        start=start,
                        stop=stop,
                    )

            def epilogue(b, psums):
                for n in range(NB):
                    osb = out_pool.tile([P, TB], FP8, tag="osb",
                                        name=f"osb_{b}_{n}")
                    nc.scalar.activation(
                        osb[:], psums[(b, n)][:], SIGN,
                        bias=bias_sb[:, b : b + 1],
                    )
                    nc.scalar.dma_start(out_d[b * P : (b + 1) * P, nsls[n]], osb[:])

            # Quad phase: blocks 0..3 k-major across all 8 PSUM banks.
            qps = {
                (b, n): psum_pool.tile([P, TB], F32, tag="psum", name=f"ps_{b}_{n}")
                for b in range(QUAD)
                for n in range(NB)
            }
            for k in range(KT):
                for n in range(NB):  # n-major: n=0 runs on half-arrived tiles
                    for b in range(QUAD):
                        if k >= JOIN_K[b]:
                            nc.tensor.matmul(
                                qps[(b, n)][:],
                                w32_tiles[b][:, k, :],
                                xk[k][:, nsls[n]],
                                start=(k == JOIN_K[b]),
                                stop=(k == KT - 1 and JOIN_K[b] == 0),
                            )
            epilogue(0, qps)
            epilogue(1, qps)
            for b in (2, 3):  # sweep the deferred heads
                for k in range(JOIN_K[b]):
                    mm(qps, b, k, start=False, stop=(k == JOIN_K[b] - 1))
                epilogue(b, qps)

            # Steady state: one block at a time, PSUM 4-deep pipelined.
            for b in range(QUAD, MT):
                stage_w8(b)
                cast_w(b)
                psums = {
                    (b, n): psum_pool.tile([P, TB], F32, tag="psum",
                                           name=f"ps_{b}_{n}")
                    for n in range(NB)
                }
                for k in range(KT):
                    mm(psums, b, k, start=(k == 0), stop=(k == KT - 1))
                epilogue(b, psums)
    nc.compile()
    _nc_cache = nc
    return nc


def prep_in_maps(x, weight, bias):
    """Host-side layout prep: transposes, fp8 sign-weight slabs, token shards."""
    x = np.asarray(x, dtype=np.float32)
    weight = np.asarray(weight, dtype=np.float32)
    bias = np.asarray(bias, dtype=np.float32)

    x_t = np.ascontiguousarray(x.T)  # [D_IN, N_TOK]
    # w8_slab[p, b, kt, m] = sign(W)[b*128+m, kt*128+p]
    w8 = np.sign(weight).astype(E4M3).reshape(MT, P, KT, P)
    w8_slab = np.ascontiguousarray(w8.transpose(3, 0, 2, 1))
    bias_t = np.ascontiguousarray(bias.reshape(MT, P).T)  # [P, MT]

    in_maps = []
    for c in range(N_CORES):
        sl = slice(c * T, (c + 1) * T)
        in_maps.append(
            {
                "x_t": np.ascontiguousarray(x_t[:, sl]),
                "w8_slab": w8_slab,
                "bias_t": bias_t,
            }
        )
    return in_maps


def run(x, weight, bias, **spmd_kwargs):
    """Run on the 8 cores; returns (full_output, BassKernelResults)."""
    nc = build()
    in_maps = prep_in_maps(x, weight, bias)
    res = run_bass_kernel_spmd(nc, in_maps, core_ids=list(range(N_CORES)), **spmd_kwargs)
    out = np.empty((N_TOK, D_OUT), dtype=np.float32)
    for c in range(N_CORES):
        out[c * T : (c + 1) * T, :] = res.results[c]["out_t"].astype(np.float32).T
    return out, res


def kernel(x, weight, bias):
    out, _ = run(x, weight, bias)
    return out
